# revision 18
# baseline (speedup 1.0000x reference)
"""
DistanceSampling Trainium2 kernel (8 NeuronCores, SPMD over patch rows).

Computation per 2x2/stride-2 patch of x (1, 256, 512, 512) fp32:
  mean over the 4 patch elements (per channel), d_k = ||x_k - mean + eps||_2
  over channels, k* = argmax_k d_k (first occurrence), out = x_{k*}.
Output: (1, 256, 65536) fp32.

Sharding: core m gets image rows [64m, 64m+64) = 32 patch rows = 8192 patch
locations; fully independent, no collectives. Output chunks concatenated on
the host along L.

Per-core design (16 qpairs of 2 patch rows x 256 cols = 512 locations),
channels on SBUF partitions (two 128-blocks in one [128, 4096] tile),
locations on the free dim:

  Squares basis: with a = x0+x1, b = x2+x3, A_k = 2*x_k - b (k=0,1) and
  B'_k = x_k - a/2 (k=2,3; the /2 compensated x4 in the W columns), the six
  pairwise distance differences (x16, eps dropped) are exact small-integer
  linear combos of the channel sums of A0^2, A1^2, B2'^2, B3'^2. Per qpair:
  one pair-sum (DVE tt, 2048), four A stt (DVE; STT APs are verifier-limited
  to 3D so no merging), sha + four B' tt (Act + GpSimd), two Square halves
  (Act), and eight PSUM-accumulating matmuls.

  Matmul dtypes: A-half squares stay exact fp32 (4 cycles/row, PE has
  headroom), B-half squares round once to f32r (~13-bit mantissa,
  1 cycle/row). This cuts PE busy from ~196us (all-fp32 baseline) to
  ~123us while keeping argmax noise to a few flips per 65536 (all-f32r:
  ~9 flips / rel 1.5e-2; this split: rel ~8e-3). fp16 squares (22 flips,
  rel 2.5e-2) fail the 2e-2 gate. The f32r producer must be the tensor's
  only writer (BIR verifier tracks rounding per tensor), hence separate
  AB2A (f32)/AB2B (f32r) tiles.

  Argmax via sign-direct beats: sg = sign(diff) on Act, beats matmul
  M (+-1 bf16) gives Ms with (Ms)_j == 3 iff j is the first-occurrence
  argmax, so the exact one-hot is relu(Ms - 2) on Act (saves the baseline's
  relu on Act and is_equal on DVE). Selection: three one-hot masks broadcast
  to 128 partitions by tiny bf16 matmuls, ot = x0 copy (Act) + three
  copy_predicated overwrites (DVE).

  X loads ride the SP ring (first load split 4 ways to shorten the ramp),
  consts and output stores ride the Act ring (a GpSimd-ring store serializes
  Pool behind the DVE selection chain, and leaves the Pool engine's f32r-ish
  rounding fingerprints on stored values). Stages are skewed across
  iterations (sign/beats at i-1; one-hot/masks/select/store at i-2) with
  per-engine emission order chosen so in-order queues never head-of-line
  block on same-iteration producers.

Measured (core 0): ~210us exec; DVE 171us busy (wall: pair-sum 43 + stt 72
+ copy_predicated 63), PE 123, Act 132, Pool 109, DMA 120 (roofline ~112).
Baseline was 218us with PE at 196-226us co-binding. Rel err ~8e-3 (a
handful of f32r-noise argmax flips at near-tie locations; gate is 2e-2).
"""

import sys

sys.path.insert(0, "/opt/trn_rl_repo")

import numpy as np

import concourse.bacc as bacc
import concourse.bass as bass
import concourse.mybir as mybir
import concourse.tile as tile
from concourse.bass_utils import run_bass_kernel_spmd

f32 = mybir.dt.float32
f32r = mybir.dt.float32r
bf16 = mybir.dt.bfloat16
Alu = mybir.AluOpType
Act = mybir.ActivationFunctionType

C, H, W = 256, 512, 512
NCORES = 8
RPC = H // NCORES  # image rows per core (64)
QP = 16  # qpair groups per core (4 image rows each)
LPC = 8192  # locations per core




def _kernel_body(tc):
    nc = tc.nc
    mm_dt = f32r  # f32r matmul runs 1 cycle/row at >=256 moving rows (vs
    # fp32's 4); squares are rounded once to f32r (~13-bit mantissa) by the
    # Act Square write: measured 5 argmax flips / 65536 (fp16 squares: 22)
    x = nc.dram_tensor("x", [C, RPC, W], f32, kind="ExternalInput").ap()
    cW = nc.dram_tensor("cW", [128, 24], mm_dt, kind="ExternalInput").ap()
    cM = nc.dram_tensor("cM", [6, 4], bf16, kind="ExternalInput").ap()
    cSEL = nc.dram_tensor("cSEL", [4, 384], bf16, kind="ExternalInput").ap()
    out = nc.dram_tensor("out", [C, LPC], f32, kind="ExternalOutput").ap()

    with (
        tc.tile_pool(name="const", bufs=1) as constp,
        tc.tile_pool(name="xin", bufs=5) as xp,
        tc.tile_pool(name="stile", bufs=2) as stp,
        tc.tile_pool(name="ab", bufs=2) as abp,
        tc.tile_pool(name="small", bufs=4) as smp,
        tc.tile_pool(name="ot", bufs=4) as otp,
        tc.tile_pool(name="ps_diff", bufs=2, space=bass.MemorySpace.PSUM) as pd,
        tc.tile_pool(name="ps_b", bufs=2, space=bass.MemorySpace.PSUM) as pb,
        tc.tile_pool(name="ps_m", bufs=1, space=bass.MemorySpace.PSUM) as pm,
    ):
        # consts ride the Act ring so the SP ring issues X loads from t=0
        W_t = constp.tile([128, 24], mm_dt)
        nc.scalar.dma_start(W_t[:], cW)
        M_t = constp.tile([6, 4], bf16)
        nc.scalar.dma_start(M_t[:], cM)
        SEL_t = constp.tile([4, 384], bf16)
        nc.scalar.dma_start(SEL_t[:], cSEL)
        neg2_t = constp.tile([4, 1], f32)
        nc.gpsimd.memset(neg2_t[:], -2.0)

        def stage_load(qp, split=1):
            # split the first load across several DMA queues so the pipeline
            # ramp is not one full 16KB/partition transfer deep
            X = xp.tile([128, 4096], f32, tag="X")
            xsrc = x.rearrange("(cb p) r w -> p cb r w", cb=2)
            xv = X[:].rearrange("p (cb r w) -> p cb r w", cb=2, r=4)
            for c in range(split):
                rr = slice(4 * c // split, 4 * (c + 1) // split)
                nc.sync.dma_start(
                    xv[:, :, rr], xsrc[:, :, 4 * qp + rr.start : 4 * qp + rr.stop, :]
                )
            return X

        def xk_view(X):
            # (cb, h, s, a, f): patch element (h, s) of location (cb, a, f)
            return X[:].rearrange(
                "p (cb a h f s) -> p cb h s a f", cb=2, a=2, h=2, s=2
            )

        def ab_view(AB):
            # (cb, k, a, f): quarter k in {A0, A1, B2, B3}
            return AB[:].rearrange("p (cb k a f) -> p cb k a f", cb=2, k=4, a=2)

        def stage_st(X):
            # one pair-sum over w-adjacent pairs for all 4 rows: both a and b
            xe = X[:].rearrange("p (q s) -> p q s", s=2)
            st = stp.tile([128, 2048], f32, tag="s")
            nc.vector.tensor_tensor(st[:], xe[:, :, 0], xe[:, :, 1], Alu.add)
            return st

        def st_view(st):
            # (cb, h, a, f): h=0 -> a (top-row sums), h=1 -> b (bottom-row)
            return st[:].rearrange("p (cb a h f) -> p cb h a f", cb=2, a=2, h=2)

        def stage_A(X, st, AB):
            # A_k = 2*x_(h=0,s=k) - b on DVE (STT APs are limited to 3D, so
            # one stt per (cb, k))
            xk = xk_view(X)
            abv = ab_view(AB)
            sv = st_view(st)
            for cb in range(2):
                for k in range(2):
                    nc.vector.scalar_tensor_tensor(
                        abv[:, cb, k], xk[:, cb, 0, k], 2.0, sv[:, cb, 1],
                        Alu.mult, Alu.subtract,
                    )

        def stage_sha(st):
            # sha = a/2 (top-row half-sums); Pool has no scalar_tensor_tensor
            # (TensorScalarPtr fails the Pool engine ISA check), so B' is a
            # plain tt subtract against sha, compensated x4 in the W columns
            sha = stp.tile([128, 1024], f32, tag="sh")
            nc.scalar.activation(
                sha[:].rearrange("p (cb a f) -> p cb a f", cb=2, a=2),
                st_view(st)[:, :, 0], Act.Copy, scale=0.5,
            )
            return sha

        def stage_B(X, sha, AB):
            # B'_k = x_(h=1,s) - a/2 on GpSimd
            xk = xk_view(X)
            abv = ab_view(AB)
            shv = sha[:].rearrange("p (cb a f) -> p cb a f", cb=2, a=2)
            for cb in range(2):
                for k in range(2):
                    nc.gpsimd.tensor_tensor(
                        abv[:, cb, 2 + k], xk[:, cb, 1, k], shv[:, cb],
                        Alu.subtract,
                    )

        def stage_sq(AB, AB2, half):
            # halves as [p, cb, 1024] 3D views: quarter pair {0,1} or {2,3}.
            # A-half squares stay exact fp32 (consumed by fp32 matmuls, PE has
            # headroom); B-half is rounded to f32r for the 1-cycle/row matmul,
            # halving the argmax noise vs all-f32r. Separate tiles per half:
            # the BIR verifier tracks f32r producer rounding per tensor.
            v = AB[:].rearrange("p (cb h q) -> p cb h q", cb=2, h=2)[:, :, half]
            v2 = AB2[half][:].rearrange("p (cb q) -> p cb q", cb=2)
            nc.scalar.activation(v2, v, Act.Square)

        def stage_mm(AB2):
            dps = pd.tile([6, 512], f32, tag="diff")
            # A-quarters (t=0,1, exact fp32) first so the matmuls only wait on
            # Sq half A, then B-quarters (t=2,3, f32r)
            order = [(0, 0), (0, 1), (1, 0), (1, 1), (0, 2), (0, 3), (1, 2), (1, 3)]
            for i, (cb, t) in enumerate(order):
                src_t = AB2[0] if t < 2 else AB2[1]
                mv = src_t[:, cb * 1024 + 512 * (t % 2) : cb * 1024 + 512 * (t % 2 + 1)]
                wv = W_t[:, 6 * t : 6 * t + 6]
                if t < 2:
                    wv = wv.bitcast(f32)
                nc.tensor.matmul(
                    dps[:], wv, mv, start=(i == 0), stop=(i == 7),
                )
            return dps

        def stage_sign(dps):
            sg = smp.tile([6, 512], bf16, tag="sg")
            nc.scalar.activation(sg[:], dps[:], Act.Sign)
            return sg

        def stage_beats(sg):
            bps = pb.tile([4, 512], f32, tag="b")
            nc.tensor.matmul(bps[:], M_t[:], sg[:], start=True, stop=True)
            return bps

        def stage_onehot(bps):
            # (Ms)_j == 3 iff j is the first-occurrence argmax; values are odd
            # so relu(Ms - 2) is the exact {0,1} one-hot
            m = smp.tile([4, 512], bf16, tag="m")
            nc.scalar.activation(m[:], bps[:], Act.Relu, bias=neg2_t[:])
            return m

        def stage_masks(m):
            masks = []
            for g in range(3):
                mk = pm.tile([128, 512], f32, tag=f"g{g}")
                nc.tensor.matmul(
                    mk[:], SEL_t[:, g * 128 : (g + 1) * 128], m[:],
                    start=True, stop=True,
                )
                masks.append(mk)
            return masks

        def stage_oinit(X):
            ot = otp.tile([128, 1024], f32, tag="o")
            nc.scalar.activation(
                ot[:].rearrange("p (cb a f) -> p cb a f", cb=2, a=2),
                xk_view(X)[:, :, 0, 0], Act.Copy,
            )
            return ot

        def stage_preds(X, masks, ot):
            xk = xk_view(X)
            ov = ot[:].rearrange("p (cb a f) -> p cb a f", cb=2, a=2)
            for g, (hk, sk) in enumerate(((0, 1), (1, 0), (1, 1))):
                mi = masks[g][:].bitcast(mybir.dt.int32).rearrange(
                    "p (a f) -> p a f", a=2
                ).unsqueeze(1).broadcast_to([128, 2, 2, 256])
                nc.vector.copy_predicated(ov, mi, xk[:, :, hk, sk])

        def stage_store(ot, qp):
            odst = out.rearrange("(cb p) l -> p cb l", cb=2)
            nc.scalar.dma_start(
                odst[:, :, qp * 512 : (qp + 1) * 512],
                ot[:].rearrange("p (cb l) -> p cb l", cb=2),
            )

        # Skewed pipeline. Emission order per iteration is arranged so each
        # in-order engine queue's head only waits on work from previous
        # iterations: Act runs (i-1)/(i-2) small ops before (i)'s Squares,
        # PE runs beats(i-1)/masks(i-2) before the (i) diff matmuls, DVE
        # runs prep before the (i-2) predicated copies, Pool runs B before
        # the (i-2) store issue.
        st_ = {}
        for i in range(QP + 2):
            if i < QP:
                d = st_[i] = {}
                d["X"] = stage_load(i, split=4 if i == 0 else 1)
                d["st"] = stage_st(d["X"])
                d["AB"] = abp.tile([128, 4096], f32, tag="AB", name="AB")
                d["AB2"] = (
                    abp.tile([128, 2048], f32, tag="AB2A", name="AB2A"),
                    abp.tile([128, 2048], mm_dt, tag="AB2B", name="AB2B"),
                )
                d["sha"] = stage_sha(d["st"])
                stage_A(d["X"], d["st"], d["AB"])
                stage_B(d["X"], d["sha"], d["AB"])
            if 1 <= i <= QP:
                st_[i - 1]["sg"] = stage_sign(st_[i - 1]["dps"])
            if 2 <= i <= QP + 1:
                st_[i - 2]["m"] = stage_onehot(st_[i - 2]["bps"])
                st_[i - 2]["ot"] = stage_oinit(st_[i - 2]["X"])
            if i < QP:
                stage_sq(d["AB"], d["AB2"], 0)
                stage_sq(d["AB"], d["AB2"], 1)
            if 1 <= i <= QP:
                st_[i - 1]["bps"] = stage_beats(st_[i - 1]["sg"])
            if 2 <= i <= QP + 1:
                q = i - 2
                st_[q]["masks"] = stage_masks(st_[q]["m"])
            if i < QP:
                d["dps"] = stage_mm(d["AB2"])
            if 2 <= i <= QP + 1:
                q = i - 2
                stage_preds(st_[q]["X"], st_[q]["masks"], st_[q]["ot"])
                stage_store(st_[q]["ot"], q)
                del st_[q]


def _const_arrays():
    import ml_dtypes

    # Delta_j = d_a - d_b (pair order (1,0),(2,0),(2,1),(3,0),(3,1),(3,2))
    # as exact linear combos of channel sums of (A0^2, A1^2, B2^2, B3^2)
    coeffs = [
        (-2, 2, 0, 0),
        (-3, -1, 12, 4),
        (-1, -3, 12, 4),
        (-3, -1, 4, 12),
        (-1, -3, 4, 12),
        (0, 0, -8, 8),
    ]
    Warr = np.zeros((128, 24), np.float32)
    for j, cf in enumerate(coeffs):
        for t in range(4):
            Warr[:, 6 * t + j] = cf[t]
    M = np.array(
        [
            [-1, 1, 0, 0],
            [-1, 0, 1, 0],
            [0, -1, 1, 0],
            [-1, 0, 0, 1],
            [0, -1, 0, 1],
            [0, 0, -1, 1],
        ],
        np.float32,
    ).astype(ml_dtypes.bfloat16)
    SEL = np.zeros((4, 384), np.float32)
    for g, k in enumerate((1, 2, 3)):
        SEL[k, g * 128 : (g + 1) * 128] = 1.0
    SEL = SEL.astype(ml_dtypes.bfloat16)
    return {"cW": Warr, "cM": M, "cSEL": SEL}


_compiled_nc = None


def _get_compiled():
    global _compiled_nc
    if _compiled_nc is None:
        nc = bacc.Bacc(
            "TRN2", target_bir_lowering=False, debug=False, num_devices=NCORES
        )
        with tile.TileContext(nc) as tc:
            _kernel_body(tc)
        nc.compile()
        _compiled_nc = nc
    return _compiled_nc


def run_sharded(x_full: np.ndarray, **spmd_kwargs):
    """x_full: (1, C, H, W) fp32. Returns (results, raw) where results is the
    assembled (1, C, L) array and raw is the BassKernelResults."""
    nc = _get_compiled()
    xs = x_full[0]  # (C, H, W)
    consts = _const_arrays()
    in_maps = [
        {"x": np.ascontiguousarray(xs[:, m * RPC : (m + 1) * RPC, :]), **consts}
        for m in range(NCORES)
    ]
    raw = run_bass_kernel_spmd(nc, in_maps, list(range(NCORES)), **spmd_kwargs)
    outs = [raw.results[m]["out"] for m in range(NCORES)]  # (C, LPC) each
    full = np.concatenate(outs, axis=1)[None]  # (1, C, L)
    return full, raw


def kernel(x: np.ndarray) -> np.ndarray:
    x = np.asarray(x, dtype=np.float32)
    assert x.shape == (1, C, H, W), x.shape
    full, _ = run_sharded(x)
    return full
